# revision 8
# baseline (speedup 1.0000x reference)
"""ComplexAttention Trainium2 kernel (8 NeuronCores, Bass/Tile).

Problem: complex-valued QKV projections + causal attention, B=4, S=2048, D=1024.
  qr,qi / kr,ki / vr,vi = complex_linear(z, W*)          (z @ W^T per component)
  scores = (qr@kr^T + qi@ki^T) / sqrt(D), causal mask, softmax
  out = stack([attn@vr, attn@vi])                        -> [2, B, S, D]

Sharding (uniform SPMD, 8 cores): core c -> (batch b = c//2, d-half = c%2).
Each core projects q/k/v for its batch restricted to its 512-wide dout half
(weights arrive host-sliced) and computes PARTIAL attention scores over that
dout half; a per-macro-block AllReduce(add) within the batch pair assembles
full scores (score compute is split, not duplicated). exp/softmax statistics
are then computed on both cores of the pair (cheap, scalar engine) and the
attention-value matmuls cover only the core's dout half.

Projections use Karatsuba complex multiplication (3 real matmuls instead of
4: M1=zr@wr, M2=zi@wi, M3=(zr+zi)@(wr+wi); yr=M1-M2, yi=M3-M1-M2); the
weight sums arrive host-computed. All matmul operands are fp16 (e5m10 - same
10-bit mantissa as fp32r at half the HBM traffic) except exp/v which are
bf16 (exp needs e8 range: max scaled score ~21 -> e^21 overflows fp16).
PSUM accumulation is fp32 throughout.

Causal masking is folded into the partial scores as an additive -30000 bias
on the 4 diagonal key blocks of each macro block (summing to -60000 after
the pair AllReduce; exp -> 0), so softmax needs no separate mask multiply.
Softmax runs without max-subtraction: denominators come from ones-matmuls
against the bf16 exp tiles.
"""

import numpy as np

B, S, D = 4, 2048, 1024
P = 128
SQ = 512  # macro block / full tile width
HC = 256  # projection half-chunk width (z tile free dim)
NDC = D // P  # 8 contraction chunks for the projections
HD = 512  # this core's dout half
HDB = HD // P  # 4 dout sub-blocks
NM = S // SQ  # 4 macro blocks
N_CORES = 8
SCALE = float(D) ** -0.5
MASKB = -30000.0  # additive causal-mask bias (doubles after AllReduce)
KBOFF = [0, 4, 12, 24]  # block-row offset of macro m in the score scratch
KBTOT = 40

_COMPILED = {}


def _build_module(reps: int = 1):
    import concourse.tile as tile
    from concourse import bacc, mybir

    f32 = mybir.dt.float32
    f16 = mybir.dt.float16
    bf16 = mybir.dt.bfloat16
    EXP = mybir.ActivationFunctionType.Exp
    AX = mybir.AxisListType.X
    ADD = mybir.AluOpType.add

    nc = bacc.Bacc("TRN2", target_bir_lowering=False, debug=False, num_devices=8)

    # ---- I/O ----
    zr_d = nc.dram_tensor("zr", [D, S], f16, kind="ExternalInput")
    zi_d = nc.dram_tensor("zi", [D, S], f16, kind="ExternalInput")
    WNAMES = ["wqr", "wqi", "wqs", "wkr", "wki", "wks", "wvr", "wvi", "wvs"]
    w_d = {n: nc.dram_tensor(n, [D, HD], f16, kind="ExternalInput") for n in WNAMES}
    dmask_d = nc.dram_tensor("dmask", [4, P, SQ], f16, kind="ExternalInput")
    o_d = nc.dram_tensor("o", [2, S, HD], f32, kind="ExternalOutput")

    # ---- DRAM scratch ----
    sp_d = nc.dram_tensor("sp", [KBTOT * P, SQ], f16, kind="Internal")
    sf_d = nc.dram_tensor("sf", [KBTOT * P, SQ], f16, kind="Internal")
    vs_d = nc.dram_tensor("vs", [2, S, HD], bf16, kind="Internal")
    PAIRS = [[0, 1], [2, 3], [4, 5], [6, 7]]

    def emit_rep(tc, pools, mask_t, ones_t, w_t):
        (zp, qp, kp, vg, cbp, spst, sfp, etp, vtp, otp, smal, pp, scp, avp) = pools

        et_t = {}  # (m, kb) -> bf16 exp tile
        recip_t = {}  # m -> [P, 4] reciprocal tile
        den_sb = {}

        def emit_proj_halfchunk(hc):
            """Project q,k,v for 256 seq positions; returns into qt/kt/vg."""
            col = (hc % 2) * HC
            zt = {}
            for nm_, src in (("zr", zr_d), ("zi", zi_d)):
                t = zp.tile([P, NDC, HC], f16, tag=nm_, bufs=2, name=nm_)
                nc.sync.dma_start(
                    t[:],
                    src[:, hc * HC : (hc + 1) * HC].rearrange("(c p) s -> p c s", p=P),
                )
                zt[nm_] = t
            zs = zp.tile([P, NDC, HC], f16, tag="zs", bufs=2, name="zs")
            nc.vector.tensor_add(
                zs.rearrange("p c s -> p (c s)"),
                zt["zr"].rearrange("p c s -> p (c s)"),
                zt["zi"].rearrange("p c s -> p (c s)"),
            )
            zt["zs"] = zs

            def karatsuba(wpre, stat_of, mov_of, out_r, out_i, width):
                """3-matmul complex combine into fp16/bf16 output slices."""
                ps = {}
                for grp, (wn, zn) in enumerate(
                    ((f"{wpre}r", "zr"), (f"{wpre}i", "zi"), (f"{wpre}s", "zs"))
                ):
                    p_ = pp.tile([P, HD], f32, tag=f"m{grp}", name=f"m{grp}")
                    for dc in range(NDC):
                        nc.tensor.matmul(
                            p_[:, 0:width],
                            stat_of(wn, zn, dc),
                            mov_of(wn, zn, dc),
                            start=(dc == 0),
                            stop=(dc == NDC - 1),
                        )
                    ps[grp] = p_
                s2 = cbp.tile([P, HD], f32, tag="s2", name="s2")
                nc.vector.tensor_copy(s2[:, 0:width], ps[1][:, 0:width])
                nc.vector.tensor_sub(out_r, ps[0][:, 0:width], s2[:, 0:width])
                u = cbp.tile([P, HD], f32, tag="u", name="u")
                nc.vector.tensor_add(u[:, 0:width], ps[0][:, 0:width], s2[:, 0:width])
                nc.vector.tensor_sub(out_i, ps[2][:, 0:width], u[:, 0:width])

            # q and k: stationary = w [128c, 128 dout], moving = z [128c, 256 s]
            for pre, pool, key in (("wq", qp, "q"), ("wk", kp, "k")):
                for db in range(HDB):
                    tiles = []
                    for ci in range(2):
                        tk = (key, hc // 2, db, ci) if key == "k" else (key, db, ci)
                        if col == 0:
                            t = pool.tile([P, SQ], f16, tag=key, name=f"{key}t")
                            _TILES[tk] = t
                        tiles.append(_TILES[tk])
                    karatsuba(
                        pre,
                        lambda wn, zn, dc: w_t[wn][:, dc, db * P : (db + 1) * P],
                        lambda wn, zn, dc: zt[zn][:, dc, :],
                        tiles[0][:, col : col + HC],
                        tiles[1][:, col : col + HC],
                        HC,
                    )
            # v: stationary = z [128c, 128 s], moving = w [128c, 512 dout]
            for sbl in range(HC // P):
                row = hc * HC + sbl * P
                vt0 = vg.tile([P, HD], bf16, tag="v", name="vt0")
                vt1 = vg.tile([P, HD], bf16, tag="v", name="vt1")
                karatsuba(
                    "wv",
                    lambda wn, zn, dc: zt[zn][:, dc, sbl * P : (sbl + 1) * P],
                    lambda wn, zn, dc: w_t[wn][:, dc, :],
                    vt0[:],
                    vt1[:],
                    HD,
                )
                nc.sync.dma_start(vs_d[0, row : row + P, :], vt0[:])
                nc.sync.dma_start(vs_d[1, row : row + P, :], vt1[:])

        def emit_scores(m):
            """Partial scores for macro m over the local dout half + AllReduce."""
            jm = 4 * m
            nkb = 4 * (m + 1)
            for kb in range(nkb):
                ps = scp.tile([P, SQ], f32, tag="sc", name="ps")
                n = 0
                for ci in range(2):
                    for dc in range(HDB):
                        nc.tensor.matmul(
                            ps[:],
                            _TILES[("k", kb // 4, dc, ci)][
                                :, (kb % 4) * P : (kb % 4 + 1) * P
                            ],
                            _TILES[("q", dc, ci)][:],
                            start=(n == 0),
                            stop=(n == 7),
                        )
                        n += 1
                st = spst.tile([P, SQ], f16, tag="st", name="st")
                if kb >= jm:
                    nc.vector.tensor_add(st[:], ps[:], mask_t[kb - jm][:])
                else:
                    nc.vector.tensor_copy(st[:], ps[:])
                nc.gpsimd.dma_start(
                    sp_d[(KBOFF[m] + kb) * P : (KBOFF[m] + kb + 1) * P, :], st[:]
                )
            blk = slice(KBOFF[m] * P, (KBOFF[m] + nkb) * P)
            nc.gpsimd.collective_compute(
                "AllReduce",
                ADD,
                replica_groups=PAIRS,
                ins=[sp_d[blk, :].opt()],
                outs=[sf_d[blk, :].opt()],
            )
            # read back + exp (gpsimd DMA queue orders naturally after the CC;
            # exp stalls only the scalar engine, which has no other work)
            for kb in range(nkb):
                sft = sfp.tile([P, SQ], f16, tag="sf", name="sft")
                nc.gpsimd.dma_start(
                    sft[:], sf_d[(KBOFF[m] + kb) * P : (KBOFF[m] + kb + 1) * P, :]
                )
                et = etp.tile([P, SQ], bf16, tag="et", name="et")
                nc.scalar.activation(et[:], sft[:], EXP)
                et_t[(m, kb)] = et

        def emit_den(m):
            """Softmax denominators for macro m (ones-matmuls + reduce)."""
            jm = 4 * m
            nkb = 4 * (m + 1)
            # den shares the score pool's PSUM ring (all 8 banks are spoken for)
            dn = scp.tile([P, 64], f32, tag="sc", name="dn")
            for kb in range(nkb):
                for sub in range(max(0, kb - jm), 4):
                    c = sub * 16 + kb
                    nc.tensor.matmul(
                        dn[:, c : c + 1],
                        et_t[(m, kb)][:, sub * P : (sub + 1) * P],
                        ones_t[:],
                        start=True,
                        stop=True,
                    )
            ds = smal.tile([P, 4], f32, tag="densb", name="ds")
            for sub in range(4):
                nc.vector.reduce_sum(
                    ds[:, sub : sub + 1],
                    dn[:, sub * 16 : sub * 16 + jm + sub + 1],
                    axis=AX,
                )
            rc = smal.tile([P, 4], f32, tag="recip", name="rc")
            nc.vector.reciprocal(rc[:], ds[:])
            den_sb[m] = ds
            recip_t[m] = rc

        def emit_av(m):
            """attn @ v for macro m over the local dout half."""
            jm = 4 * m
            for ci in range(2):
                vtd = {}
                for pass_subs in ((0, 1, 2), (3,)):
                    hi = jm + pass_subs[-1]
                    for kb in range(hi + 1):
                        if kb not in vtd:
                            vt = vtp.tile([P, HD], bf16, tag="vt", name="vt")
                            nc.sync.dma_start(
                                vt[:], vs_d[ci, kb * P : (kb + 1) * P, :]
                            )
                            vtd[kb] = vt
                        for sub in pass_subs:
                            j = jm + sub
                            if kb > j:
                                continue
                            tag = f"a{sub % 3}"
                            if kb == 0:
                                _TILES[("av", sub)] = avp.tile(
                                    [P, HD], f32, tag=tag, name=tag
                                )
                            nc.tensor.matmul(
                                _TILES[("av", sub)][:],
                                et_t[(m, kb)][:, sub * P : (sub + 1) * P],
                                vtd[kb][:],
                                start=(kb == 0),
                                stop=(kb == j),
                            )
                    for sub in pass_subs:
                        ot = otp.tile([P, HD], f32, tag="ot", name="ot")
                        nc.vector.tensor_scalar_mul(
                            ot[:], _TILES[("av", sub)][:], recip_t[m][:, sub : sub + 1]
                        )
                        nc.sync.dma_start(
                            o_d[ci, m * SQ + sub * P : m * SQ + (sub + 1) * P, :],
                            ot[:],
                        )

        _TILES = {}
        # ---- main pipeline: proj(m) -> scores(m) -> AllReduce(m) ----
        for m in range(NM):
            emit_proj_halfchunk(2 * m)
            emit_proj_halfchunk(2 * m + 1)
            emit_scores(m)
        # ---- post phase: dens + AV ordered to hide AllReduce(3) latency ----
        emit_den(0)
        emit_den(1)
        emit_av(0)
        emit_den(2)
        emit_av(1)
        emit_den(3)
        emit_av(2)
        emit_av(3)

    with tile.TileContext(nc) as tc:
        with (
            tc.tile_pool(name="const", bufs=1) as cp,
            tc.tile_pool(name="wp", bufs=1) as wp,
            tc.tile_pool(name="zp", bufs=2) as zp,
            tc.tile_pool(name="qp", bufs=12) as qp,
            tc.tile_pool(name="kp", bufs=32) as kp,
            tc.tile_pool(name="vg", bufs=4) as vg,
            tc.tile_pool(name="cbp", bufs=2) as cbp,
            tc.tile_pool(name="spst", bufs=4) as spst,
            tc.tile_pool(name="sfp", bufs=4) as sfp,
            tc.tile_pool(name="etp", bufs=18) as etp,
            tc.tile_pool(name="vtp", bufs=16) as vtp,
            tc.tile_pool(name="otp", bufs=4) as otp,
            tc.tile_pool(name="smal", bufs=4) as smal,
            tc.tile_pool(name="pp", bufs=1, space="PSUM") as pp,
            tc.tile_pool(name="scp", bufs=2, space="PSUM") as scp,
            tc.tile_pool(name="avp", bufs=1, space="PSUM") as avp,
        ):
            mask_t = []
            for idx in range(4):
                mt = cp.tile([P, SQ], f16, tag=f"mask{idx}", name=f"mask{idx}")
                nc.sync.dma_start(mt[:], dmask_d[idx])
                mask_t.append(mt)
            ones_t = cp.tile([P, 1], bf16, tag="ones", name="ones_t")
            nc.vector.memset(ones_t[:], 1.0)
            w_t = {}
            for n in ["wqr", "wqi", "wqs", "wkr", "wki", "wks", "wvr", "wvi", "wvs"]:
                t = wp.tile([P, NDC, HD], f16, tag=n, name=n)
                nc.sync.dma_start(t[:], w_d[n].rearrange("(c p) h -> p c h", p=P))
                w_t[n] = t
            pools = (
                zp, qp, kp, vg, cbp, spst, sfp, etp, vtp, otp, smal,
                pp, scp, avp,
            )
            for _rep in range(reps):
                emit_rep(tc, pools, mask_t, ones_t, w_t)

    nc.compile()
    return nc


def get_module(reps: int = 1):
    key = ("nc", reps)
    if key not in _COMPILED:
        _COMPILED[key] = _build_module(reps)
    return _COMPILED[key]


def prepare_in_maps(z_real, z_imag, wq_r, wq_i, wk_r, wk_i, wv_r, wv_i, mask):
    """Host-side sharding/layout prep -> list of per-core input dicts."""
    f16 = np.float16
    zT_r = [np.asarray(z_real)[b].T.astype(f16) for b in range(B)]
    zT_i = [np.asarray(z_imag)[b].T.astype(f16) for b in range(B)]
    # torch Linear W is [out, in]; matmuls want W^T = [in, out].
    # The q weights absorb the 1/sqrt(D) score scale.
    w16 = {
        "wqr": (np.asarray(wq_r).T * SCALE).astype(f16),
        "wqi": (np.asarray(wq_i).T * SCALE).astype(f16),
        "wkr": np.asarray(wk_r).T.astype(f16),
        "wki": np.asarray(wk_i).T.astype(f16),
        "wvr": np.asarray(wv_r).T.astype(f16),
        "wvi": np.asarray(wv_i).T.astype(f16),
    }
    for pre in ("wq", "wk", "wv"):
        w16[f"{pre}s"] = (
            w16[f"{pre}r"].astype(np.float32) + w16[f"{pre}i"].astype(np.float32)
        ).astype(f16)
    # diagonal-block additive mask bias from the provided mask (macro 3 rows)
    mask = np.asarray(mask)
    dmask = np.zeros((4, P, SQ), f16)
    g0 = 3 * SQ
    for idx in range(4):
        k0 = (12 + idx) * P
        keep = (mask[g0 : g0 + SQ, k0 : k0 + P] != 0).T
        dmask[idx] = np.where(keep, np.float16(0.0), np.float16(MASKB))
    in_maps = []
    for c in range(N_CORES):
        b, dh = c // 2, c % 2
        half = slice(dh * HD, (dh + 1) * HD)
        im = {"zr": zT_r[b], "zi": zT_i[b], "dmask": dmask}
        for n, w in w16.items():
            im[n] = np.ascontiguousarray(w[:, half])
        in_maps.append(im)
    return in_maps


def assemble_output(results):
    """Per-core outputs [2, S, 512] -> full [2, B, S, D]."""
    out = np.empty((2, B, S, D), np.float32)
    for c in range(N_CORES):
        b, dh = c // 2, c % 2
        out[:, b, :, dh * HD : (dh + 1) * HD] = results[c]["o"]
    return out


def kernel(**inputs) -> np.ndarray:
    from concourse.bass_utils import run_bass_kernel_spmd

    nc = get_module()
    in_maps = prepare_in_maps(**inputs)
    res = run_bass_kernel_spmd(nc, in_maps, core_ids=list(range(N_CORES)))
    return assemble_output(res.results)
